# revision 5
# baseline (speedup 1.0000x reference)
"""BiSSM (bidirectional Mamba block) Trainium2 kernel.

Contract: kernel(**inputs) takes the FULL unsharded inputs of
nn_BiSSMBlock (see setup_inputs) and returns the full (2, 1024, 1024)
output.  Internally shards across 8 NeuronCores as
(batch 2) x (direction 2) x (d_inner half 2); each core runs an identical
Bass program on host-sliced data, with a pairwise AllReduce between
half-cores for the x_dbl projection.  Host folds Wout into proj_W
(per direction) and sums/flips partials.

Self-contained: only needs the concourse/bass toolchain at
/opt/trn_rl_repo and 8 visible neuron cores.
"""
import sys
sys.path.insert(0, "/opt/trn_rl_repo")
import numpy as np

import concourse.bass as bass
import concourse.bacc as bacc
import concourse.mybir as mybir
import concourse.tile as tile

F32 = mybir.dt.float32
F32R = mybir.dt.float32r
BF16 = mybir.dt.bfloat16
OP = mybir.AluOpType
AF = mybir.ActivationFunctionType

L = 1024          # sequence length
DM = 1024         # d_model
DH = 1024         # d_inner half per core
NG = 8            # channel segments (DH/128)
NST = 16          # d_state
NT = 2            # time chunks of 512 for matmul moving dim
TN = 512

N_CORES = 8
CC_GROUPS = [[0, 1], [2, 3], [4, 5], [6, 7]]


def _build(structured_a, g_blk=4, scan_dt=BF16, xin_dt=BF16, n_cores=N_CORES):
    nc = bacc.Bacc("TRN2", target_bir_lowering=False, debug=False, num_devices=n_cores)
    NB = NG // g_blk
    FB = g_blk * L

    xT = nc.declare_dram_parameter("xT", [DM, L], F32, isOutput=False)
    w_in = nc.declare_dram_parameter("w_in", [DM, 2 * DH], F32, isOutput=False)
    wx = nc.declare_dram_parameter("wx", [DH, 96], F32, isOutput=False)
    wdt = nc.declare_dram_parameter("wdt", [64, DH], F32, isOutput=False)
    wco = nc.declare_dram_parameter("wco", [DH, DM], F32, isOutput=False)
    convw = nc.declare_dram_parameter("convw", [128, NG, 4], F32, isOutput=False)
    convb = nc.declare_dram_parameter("convb", [128, NG], F32, isOutput=False)
    binz = nc.declare_dram_parameter("binz", [128, NG], F32, isOutput=False)
    bdt = nc.declare_dram_parameter("bdt", [128, NG], F32, isOutput=False)
    A_ = nc.declare_dram_parameter("A_", [128, NG, NST], F32, isOutput=False)
    dsk = nc.declare_dram_parameter("dsk", [128, NG], F32, isOutput=False)
    outp = nc.declare_dram_parameter("outp", [DM, L], F32, isOutput=True)

    cc_in = nc.dram_tensor("cc_in", [96, L], F32)
    cc_out = nc.dram_tensor("cc_out", [96, L], F32)

    with tile.TileContext(nc) as tc:
        consts_cm = tc.tile_pool(name="consts", bufs=1)
        consts = consts_cm.__enter__()
        cw = consts.tile([128, NG, 4], F32)
        nc.sync.dma_start(out=cw[:], in_=convw[:])
        cb = consts.tile([128, NG], F32)
        nc.sync.dma_start(out=cb[:], in_=convb[:])
        bz = consts.tile([128, NG], F32)
        nc.sync.dma_start(out=bz[:], in_=binz[:])
        bd = consts.tile([128, NG], F32)
        nc.sync.dma_start(out=bd[:], in_=bdt[:])
        At = consts.tile([128, NG, NST], F32)
        nc.sync.dma_start(out=At[:], in_=A_[:])
        dk = consts.tile([128, NG], F32)
        nc.sync.dma_start(out=dk[:], in_=dsk[:])

        yacp_cm = tc.tile_pool(name="yacp", bufs=1)
        yacp = yacp_cm.__enter__()
        yac = yacp.tile([128, NG, L], F32, tag="yac")

        szgp_cm = tc.tile_pool(name="szgp", bufs=1)
        szgp = szgp_cm.__enter__()
        szg = szgp.tile([128, NG, L], scan_dt, tag="szg")

        poolD_cm = tc.tile_pool(name="poolD", bufs=1)
        poolD = poolD_cm.__enter__()
        delta = poolD.tile([128, NG, L], F32, tag="delta")
        du = poolD.tile([128, NG, L], scan_dt, tag="du")

        poolU_cm = tc.tile_pool(name="poolU", bufs=1)
        poolU = poolU_cm.__enter__()
        xin = poolU.tile([128, NG, L + 3], xin_dt, tag="xin")
        u3 = poolU.tile([128, NG, L], BF16, tag="u3")
        halo = bass.AP(tensor=xin.tensor, offset=xin.offset,
                       ap=[[xin.ap[0][0], 128], [L + 3, NG], [1, 3]])
        nc.vector.memset(halo, 0.0)

        w_in_r = w_in.ap().rearrange("(k p) m -> p k m", p=128).bitcast(F32R)

        # ---------------- Phase A1: in_proj (z first, then xin) ----------------
        with tc.tile_pool(name="poolW", bufs=1) as poolW, \
             tc.tile_pool(name="wchunk", bufs=2) as wchunk, \
             tc.tile_pool(name="mm_ps", bufs=4, space="PSUM") as mm_ps:
            xts = poolW.tile([128, 8, L], F32R, tag="xts")
            nc.sync.dma_start(out=xts[:], in_=xT.ap().rearrange("(k p) t -> p k t", p=128).bitcast(F32R))
            for mc in range(16):
                m = mc % 8
                is_z = mc < 8
                col0 = (DH if is_z else 0) + m * 128
                wi = wchunk.tile([128, 8, 128], F32R, tag="wi")
                nc.sync.dma_start(out=wi[:], in_=w_in_r[:, :, col0:col0 + 128])
                for tn in range(NT):
                    ps = mm_ps.tile([128, TN], F32, tag="ps")
                    for k in range(8):
                        nc.tensor.matmul(ps[:], wi[:, k, :],
                                         xts[:, k, tn * TN:(tn + 1) * TN],
                                         start=(k == 0), stop=(k == 7))
                    if is_z:
                        nc.scalar.activation(out=szg[:, m, tn * TN:(tn + 1) * TN],
                                             in_=ps[:], func=AF.Silu,
                                             bias=bz[:, m:m + 1], scale=1.0)
                    else:
                        nc.scalar.copy(out=xin[:, m, 3 + tn * TN: 3 + (tn + 1) * TN], in_=ps[:])

            # conv (fp32 scratch per segment) + silu -> u (bf16)
            for g in range(NG):
                scr = wchunk.tile([128, L], F32, tag="scr")
                nc.vector.tensor_scalar_mul(out=scr[:], in0=xin[:, g, 3:3 + L],
                                            scalar1=cw[:, g, 3:4])
                for k in range(3):
                    nc.vector.scalar_tensor_tensor(
                        out=scr[:], in0=xin[:, g, k:k + L],
                        scalar=cw[:, g, k:k + 1], in1=scr[:],
                        op0=OP.mult, op1=OP.add)
                nc.scalar.activation(out=u3[:, g, :], in_=scr[:], func=AF.Silu,
                                     bias=cb[:, g:g + 1], scale=1.0)
                nc.vector.tensor_scalar_mul(out=yac[:, g, :], in0=u3[:, g, :], scalar1=dk[:, g:g + 1])

        # ---------------- Phase A2: Wx partial + AllReduce + delta ----------------
        with tc.tile_pool(name="wxp", bufs=1) as wxp, \
             tc.tile_pool(name="wx_ps", bufs=4, space="PSUM") as wx_ps:
            wxs = wxp.tile([128, 8, 96], BF16, tag="wxs")
            nc.gpsimd.dma_start(out=wxs[:], in_=wx.ap().rearrange("(k p) m -> p k m", p=128))
            xdb = wxp.tile([96, L], F32, tag="xdb")
            for tn in range(NT):
                ps96 = wx_ps.tile([96, TN], F32, tag="ps96")
                for k in range(8):
                    nc.tensor.matmul(ps96[:], wxs[:, k, :],
                                     u3[:, k, tn * TN:(tn + 1) * TN],
                                     start=(k == 0), stop=(k == 7))
                nc.vector.tensor_copy(out=xdb[:, tn * TN:(tn + 1) * TN], in_=ps96[:])
            nc.sync.dma_start(out=cc_in[:], in_=xdb[:])
            nc.gpsimd.collective_compute("AllReduce", OP.add, replica_groups=CC_GROUPS,
                                         ins=[cc_in[:]], outs=[cc_out[:]])

            dts = wxp.tile([64, L], F32R, tag="dts")
            nc.sync.dma_start(out=dts[:], in_=cc_out[0:64, :].bitcast(F32R))
            wds = wxp.tile([64, NG, 128], F32R, tag="wds")
            nc.sync.dma_start(out=wds[:], in_=wdt.ap().rearrange("k (g p) -> k g p", p=128).bitcast(F32R))
            for g in range(NG):
                for tn in range(NT):
                    psd = wx_ps.tile([128, TN], F32, tag="psd")
                    nc.tensor.matmul(psd[:], wds[:, g, :], dts[:, tn * TN:(tn + 1) * TN],
                                     start=True, stop=True)
                    dsl = delta[:, g, tn * TN:(tn + 1) * TN]
                    nc.scalar.activation(out=dsl, in_=psd[:],
                                         func=AF.Exp, bias=bd[:, g:g + 1], scale=1.0)
                    nc.scalar.activation(out=dsl, in_=dsl,
                                         func=AF.Ln, bias=1.0, scale=1.0)

            nc.vector.tensor_tensor(out=du[:], in0=delta[:], in1=u3[:], op=OP.mult)
        poolU_cm.__exit__(None, None, None)

        # ---------------- Phase B: selective scan ----------------
        with tc.tile_pool(name="scan", bufs=2) as sp, \
             tc.tile_pool(name="scan3", bufs=3) as sp3:
            quads = {}
            for n in range(NST):
                brep = sp3.tile([128, L], scan_dt, tag="brep")
                nc.gpsimd.dma_start(out=brep[:], in_=cc_out[64 + n: 65 + n, :].to_broadcast((128, L)))
                crep = sp3.tile([128, L], scan_dt, tag="crep")
                nc.gpsimd.dma_start(out=crep[:], in_=cc_out[80 + n: 81 + n, :].to_broadcast((128, L)))
                for blk in range(NB):
                    g0 = blk * g_blk
                    dA = sp.tile([128, FB], scan_dt, tag="dA")
                    if structured_a:
                        nc.scalar.activation(
                            out=dA[:].rearrange("p (g t) -> p g t", g=g_blk),
                            in_=delta[:, g0:g0 + g_blk, :],
                            func=AF.Exp, bias=0.0, scale=At[:, 0, n:n + 1])
                    else:
                        for gg in range(g_blk):
                            nc.scalar.activation(
                                out=dA[:, gg * L:(gg + 1) * L],
                                in_=delta[:, g0 + gg, :],
                                func=AF.Exp, bias=0.0, scale=At[:, g0 + gg, n:n + 1])
                    dAz = bass.AP(tensor=dA.tensor, offset=dA.offset,
                                  ap=[[dA.ap[0][0], 128], [L, g_blk], [1, 1]])
                    nc.gpsimd.memset(dAz, 0.0)
                    dBu = sp.tile([128, FB], scan_dt, tag="dBu")
                    bap = bass.AP(tensor=brep.tensor, offset=brep.offset,
                                  ap=[[brep.ap[0][0], 128], [0, g_blk], [1, L]])
                    dbu_eng = nc.gpsimd if (n % 2 == 0) else nc.vector
                    dbu_eng.tensor_tensor(
                        out=dBu[:].rearrange("p (g t) -> p g t", g=g_blk),
                        in0=du[:, g0:g0 + g_blk, :], in1=bap, op=OP.mult)
                    h = sp.tile([128, FB], scan_dt, tag="h")
                    nc.vector.tensor_tensor_scan(h[:], dA[:], dBu[:], 0.0, OP.mult, OP.add)
                    hc = sp.tile([128, FB], scan_dt, tag="hc")
                    cap = bass.AP(tensor=crep.tensor, offset=crep.offset,
                                  ap=[[crep.ap[0][0], 128], [0, g_blk], [1, L]])
                    hc_eng = nc.gpsimd if (n % 4 in (1, 3)) else nc.vector
                    hc_eng.tensor_tensor(
                        out=hc[:].rearrange("p (g t) -> p g t", g=g_blk),
                        in0=h[:].rearrange("p (g t) -> p g t", g=g_blk), in1=cap, op=OP.mult)
                    ph = n % 4
                    if ph == 0:
                        quad = sp.tile([128, FB], scan_dt, tag="quad")
                        quads[blk] = quad
                        nc.gpsimd.tensor_copy(quad[:], hc[:])
                    else:
                        quad = quads[blk]
                        nc.vector.tensor_tensor(out=quad[:], in0=quad[:], in1=hc[:], op=OP.add)
                    if ph == 3:
                        nc.vector.tensor_tensor(
                            out=yac[:, g0:g0 + g_blk, :],
                            in0=yac[:, g0:g0 + g_blk, :],
                            in1=quad[:].rearrange("p (g t) -> p g t", g=g_blk), op=OP.add)
        poolD_cm.__exit__(None, None, None)

        # ---------------- gate + out projection ----------------
        with tc.tile_pool(name="oproj", bufs=1) as op_pool, \
             tc.tile_pool(name="oc", bufs=2) as oc_pool, \
             tc.tile_pool(name="op_ps", bufs=4, space="PSUM") as op_ps:
            ygb = op_pool.tile([128, NG, L], BF16, tag="ygb")
            nc.vector.tensor_tensor(out=ygb[:], in0=yac[:], in1=szg[:], op=OP.mult)
            osb = op_pool.tile([128, 8, L], F32, tag="osb")
            wco_r = wco.ap().rearrange("(k p) m -> p k m", p=128)
            outp_r = outp.ap().rearrange("(m p) t -> p m t", p=128)
            for mc in range(2):
                wc = oc_pool.tile([128, 8, 512], BF16, tag="wc")
                nc.gpsimd.dma_start(out=wc[:], in_=wco_r[:, :, mc * 512:(mc + 1) * 512])
                for mm in range(4):
                    m = mc * 4 + mm
                    for tn in range(NT):
                        ps = op_ps.tile([128, TN], F32, tag="ps_o")
                        for k in range(8):
                            nc.tensor.matmul(ps[:], wc[:, k, mm * 128:(mm + 1) * 128],
                                             ygb[:, k, tn * TN:(tn + 1) * TN],
                                             start=(k == 0), stop=(k == 7))
                        nc.scalar.copy(out=osb[:, m, tn * TN:(tn + 1) * TN], in_=ps[:])
                    nc.sync.dma_start(out=outp_r[:, m, :], in_=osb[:, m, :])
        szgp_cm.__exit__(None, None, None)
        yacp_cm.__exit__(None, None, None)
        consts_cm.__exit__(None, None, None)

    nc.compile()
    return nc


def _prep_core_inputs(inputs, b, d, h):
    pref = "f_" if d == 0 else "b_"
    g = lambda k: np.asarray(inputs[pref + k], dtype=np.float32)
    x = np.asarray(inputs["x"], dtype=np.float32)[b]
    if d == 1:
        x = x[::-1]
    sl = slice(h * DH, (h + 1) * DH)

    Win = g("Win")
    w_in = np.concatenate([Win[sl].T, Win[2048 + h * DH: 2048 + (h + 1) * DH].T], axis=1)
    convw = g("convw")[sl]
    convb_eff = g("convb")[sl] + g("bin")[sl] * convw.sum(-1)
    pg = lambda v: np.ascontiguousarray(v.reshape(NG, 128).T)
    pg3 = lambda v: np.ascontiguousarray(v.reshape(NG, 128, -1).transpose(1, 0, 2))
    A = -np.exp(g("Alog")[sl])
    proj_W = np.asarray(inputs["proj_W"], dtype=np.float32)
    Pd = proj_W[:, d * DM:(d + 1) * DM]
    wco = (Pd @ g("Wout"))[:, sl].T
    return {
        "xT": np.ascontiguousarray(x.T),
        "w_in": np.ascontiguousarray(w_in),
        "wx": np.ascontiguousarray(g("Wx")[:, sl].T),
        "wdt": np.ascontiguousarray(g("Wdt")[sl].T),
        "wco": np.ascontiguousarray(wco),
        "convw": pg3(convw),
        "convb": pg(convb_eff),
        "binz": pg(g("bin")[2048 + h * DH: 2048 + (h + 1) * DH]),
        "bdt": pg(g("bdt")[sl]),
        "A_": pg3(A),
        "dsk": pg(g("Dsk")[sl]),
    }


def _check_structured_a(inputs):
    ar = np.log(np.arange(1, NST + 1, dtype=np.float32))
    for pref in ("f_", "b_"):
        Alog = np.asarray(inputs[pref + "Alog"], dtype=np.float32)
        if not np.allclose(Alog, np.broadcast_to(ar, Alog.shape), atol=1e-5):
            return False
    return True


_CACHE = {}


def _get_nc(structured_a):
    key = ("v1", structured_a)
    if key not in _CACHE:
        _CACHE[key] = _build(structured_a)
    return _CACHE[key]


def kernel(**inputs):
    from concourse.bass_utils import run_bass_kernel_spmd

    nc = _get_nc(_check_structured_a(inputs))
    in_maps = []
    for c in range(N_CORES):
        b, d, h = c >> 2, (c >> 1) & 1, c & 1
        in_maps.append(_prep_core_inputs(inputs, b, d, h))
    res = run_bass_kernel_spmd(nc, in_maps, list(range(N_CORES)))
    partials = [res.results[c]["outp"] for c in range(N_CORES)]

    B = 2
    out = np.zeros((B, L, DM), np.float32)
    for b in range(B):
        for d in range(2):
            s = (partials[b * 4 + d * 2 + 0] + partials[b * 4 + d * 2 + 1]).T
            if d == 1:
                s = s[::-1]
            out[b] += s
    proj_W = np.asarray(inputs["proj_W"], dtype=np.float32)
    bias = (np.asarray(inputs["f_bout"], dtype=np.float32) @ proj_W[:, :DM].T
            + np.asarray(inputs["b_bout"], dtype=np.float32) @ proj_W[:, DM:].T
            + np.asarray(inputs["proj_b"], dtype=np.float32))
    return out + bias


# revision 10
# speedup vs baseline: 1.6676x; 1.6676x over previous
"""BiSSM (bidirectional Mamba block) Trainium2 kernel.

Contract: kernel(**inputs) takes the FULL unsharded inputs of
nn_BiSSMBlock (see setup_inputs) and returns the full (2, 1024, 1024)
output.  Internally shards across 8 NeuronCores as
(batch 2) x (direction 2) x (d_inner half 2); each core runs an identical
Bass program on host-sliced data, with a pairwise AllReduce between
half-cores for the x_dbl projection.  Host folds Wout into proj_W
(per direction) and sums/flips partials.

Self-contained: only needs the concourse/bass toolchain at
/opt/trn_rl_repo and 8 visible neuron cores.
"""
import sys
sys.path.insert(0, "/opt/trn_rl_repo")
import numpy as np

import concourse.bass as bass
import concourse.bacc as bacc
import concourse.mybir as mybir
import concourse.tile as tile

F32 = mybir.dt.float32
F32R = mybir.dt.float32r
BF16 = mybir.dt.bfloat16
OP = mybir.AluOpType
AF = mybir.ActivationFunctionType

L = 1024          # sequence length
DM = 1024         # d_model
DH = 1024         # d_inner half per core
NG = 8            # channel segments (DH/128)
NST = 16          # d_state
NT = 2            # time chunks of 512 for matmul moving dim
TN = 512

N_CORES = 8
CC_GROUPS = [[0, 1], [2, 3], [4, 5], [6, 7]]


def _build(structured_a, g_blk=4, scan_dt=BF16, xin_dt=BF16, n_cores=N_CORES):
    nc = bacc.Bacc("TRN2", target_bir_lowering=False, debug=False, num_devices=n_cores)
    NB = NG // g_blk
    FB = g_blk * L

    xT = nc.declare_dram_parameter("xT", [DM, L], F32, isOutput=False)
    w_in = nc.declare_dram_parameter("w_in", [DM, 2 * DH], F32, isOutput=False)
    wx = nc.declare_dram_parameter("wx", [DH, 96], F32, isOutput=False)
    wdt = nc.declare_dram_parameter("wdt", [64, DH], F32, isOutput=False)
    wco = nc.declare_dram_parameter("wco", [DH, DM], F32, isOutput=False)
    convw = nc.declare_dram_parameter("convw", [128, NG, 4], F32, isOutput=False)
    convb = nc.declare_dram_parameter("convb", [128, NG], F32, isOutput=False)
    binz = nc.declare_dram_parameter("binz", [128, NG], F32, isOutput=False)
    bdt = nc.declare_dram_parameter("bdt", [128, NG], F32, isOutput=False)
    A_ = nc.declare_dram_parameter("A_", [128, NG, NST], F32, isOutput=False)
    dsk = nc.declare_dram_parameter("dsk", [128, NG], F32, isOutput=False)
    outp = nc.declare_dram_parameter("outp", [2, DM, L], F32, isOutput=True)

    cc_in = nc.dram_tensor("cc_in", [96, L], F32)
    cc_out = nc.dram_tensor("cc_out", [96, L], F32)
    cc_bf = nc.dram_tensor("cc_bf", [32, L], BF16)

    with tile.TileContext(nc) as tc:
        consts_cm = tc.tile_pool(name="consts", bufs=1)
        consts = consts_cm.__enter__()
        cw = consts.tile([128, NG, 4], F32)
        nc.sync.dma_start(out=cw[:], in_=convw[:])
        cb = consts.tile([128, NG], F32)
        nc.sync.dma_start(out=cb[:], in_=convb[:])
        bz = consts.tile([128, NG], F32)
        nc.sync.dma_start(out=bz[:], in_=binz[:])
        bd = consts.tile([128, NG], F32)
        nc.sync.dma_start(out=bd[:], in_=bdt[:])
        At = consts.tile([128, NG, NST], F32)
        nc.sync.dma_start(out=At[:], in_=A_[:])
        dk = consts.tile([128, NG], F32)
        nc.sync.dma_start(out=dk[:], in_=dsk[:])

        yacp_cm = tc.tile_pool(name="yacp", bufs=1)
        yacp = yacp_cm.__enter__()
        yac = yacp.tile([128, NG, L], F32, tag="yac")

        szgp_cm = tc.tile_pool(name="szgp", bufs=1)
        szgp = szgp_cm.__enter__()
        szg = szgp.tile([128, NG, L], scan_dt, tag="szg")

        poolD_cm = tc.tile_pool(name="poolD", bufs=1)
        poolD = poolD_cm.__enter__()
        delta = poolD.tile([128, NG, L], F32, tag="delta")
        du = poolD.tile([128, NG, L], scan_dt, tag="du")

        poolU_cm = tc.tile_pool(name="poolU", bufs=1)
        poolU = poolU_cm.__enter__()
        xin = poolU.tile([128, NG, L + 3], xin_dt, tag="xin")
        u3 = poolU.tile([128, NG, L], BF16, tag="u3")
        halo = bass.AP(tensor=xin.tensor, offset=xin.offset,
                       ap=[[xin.ap[0][0], 128], [L + 3, NG], [1, 3]])
        nc.vector.memset(halo, 0.0)

        w_in_r = w_in.ap().rearrange("(k p) m -> p k m", p=128).bitcast(F32R)

        # ---------------- Phase A1: in_proj (z first, then xin) ----------------
        with tc.tile_pool(name="poolW", bufs=1) as poolW, \
             tc.tile_pool(name="wchunk", bufs=2) as wchunk, \
             tc.tile_pool(name="mm_ps", bufs=4, space="PSUM") as mm_ps:
            xts = poolW.tile([128, 8, L], F32R, tag="xts")
            nc.sync.dma_start(out=xts[:], in_=xT.ap().rearrange("(k p) t -> p k t", p=128).bitcast(F32R))
            for mc in range(16):
                m = mc % 8
                is_z = mc >= 8
                col0 = (DH if is_z else 0) + m * 128
                wi = wchunk.tile([128, 8, 128], F32R, tag="wi")
                nc.sync.dma_start(out=wi[:], in_=w_in_r[:, :, col0:col0 + 128])
                for tn in range(NT):
                    ps = mm_ps.tile([128, TN], F32, tag="ps")
                    for k in range(8):
                        nc.tensor.matmul(ps[:], wi[:, k, :],
                                         xts[:, k, tn * TN:(tn + 1) * TN],
                                         start=(k == 0), stop=(k == 7))
                    if is_z:
                        nc.scalar.activation(out=szg[:, m, tn * TN:(tn + 1) * TN],
                                             in_=ps[:], func=AF.Silu,
                                             bias=bz[:, m:m + 1], scale=1.0)
                    else:
                        nc.scalar.copy(out=xin[:, m, 3 + tn * TN: 3 + (tn + 1) * TN], in_=ps[:])

            # conv (fp32 scratch per segment) + silu -> u (bf16)
            for g in range(NG):
                scr = wchunk.tile([128, L], F32, tag="scr")
                nc.vector.tensor_scalar_mul(out=scr[:], in0=xin[:, g, 3:3 + L],
                                            scalar1=cw[:, g, 3:4])
                for k in range(3):
                    nc.vector.scalar_tensor_tensor(
                        out=scr[:], in0=xin[:, g, k:k + L],
                        scalar=cw[:, g, k:k + 1], in1=scr[:],
                        op0=OP.mult, op1=OP.add)
                nc.scalar.activation(out=u3[:, g, :], in_=scr[:], func=AF.Silu,
                                     bias=cb[:, g:g + 1], scale=1.0)
                nc.vector.tensor_scalar_mul(out=yac[:, g, :], in0=u3[:, g, :], scalar1=dk[:, g:g + 1])

        # ---------------- Phase A2: Wx partial + AllReduce + delta ----------------
        with tc.tile_pool(name="wxp", bufs=1) as wxp, \
             tc.tile_pool(name="wx_ps", bufs=4, space="PSUM") as wx_ps:
            wxs = wxp.tile([128, 8, 96], BF16, tag="wxs")
            nc.gpsimd.dma_start(out=wxs[:], in_=wx.ap().rearrange("(k p) m -> p k m", p=128))
            xdb = wxp.tile([96, L], F32, tag="xdb")
            for tn in range(NT):
                ps96 = wx_ps.tile([96, TN], F32, tag="ps96")
                for k in range(8):
                    nc.tensor.matmul(ps96[:], wxs[:, k, :],
                                     u3[:, k, tn * TN:(tn + 1) * TN],
                                     start=(k == 0), stop=(k == 7))
                nc.vector.tensor_copy(out=xdb[:, tn * TN:(tn + 1) * TN], in_=ps96[:])
            nc.sync.dma_start(out=cc_in[:], in_=xdb[:])
            nc.gpsimd.collective_compute("AllReduce", OP.add, replica_groups=CC_GROUPS,
                                         ins=[cc_in[:]], outs=[cc_out[:]])
            nc.gpsimd.dma_start(out=cc_bf[:], in_=cc_out[64:96, :])

            dts = wxp.tile([64, L], F32R, tag="dts")
            nc.sync.dma_start(out=dts[:], in_=cc_out[0:64, :].bitcast(F32R))
            wds = wxp.tile([64, NG, 128], F32R, tag="wds")
            nc.sync.dma_start(out=wds[:], in_=wdt.ap().rearrange("k (g p) -> k g p", p=128).bitcast(F32R))
            for g in range(NG):
                for tn in range(NT):
                    psd = wx_ps.tile([128, TN], F32, tag="psd")
                    nc.tensor.matmul(psd[:], wds[:, g, :], dts[:, tn * TN:(tn + 1) * TN],
                                     start=True, stop=True)
                    dsl = delta[:, g, tn * TN:(tn + 1) * TN]
                    nc.scalar.activation(out=dsl, in_=psd[:],
                                         func=AF.Exp, bias=bd[:, g:g + 1], scale=1.0)
                if g == 3:
                    nc.scalar.activation(out=delta[:, 0:4, :], in_=delta[:, 0:4, :],
                                         func=AF.Ln, bias=1.0, scale=1.0)
                    nc.gpsimd.tensor_tensor(out=du[:, 0:4, :], in0=delta[:, 0:4, :],
                                            in1=u3[:, 0:4, :], op=OP.mult)
            nc.scalar.activation(out=delta[:, 4:8, :], in_=delta[:, 4:8, :],
                                 func=AF.Ln, bias=1.0, scale=1.0)
            nc.gpsimd.tensor_tensor(out=du[:, 4:8, :], in0=delta[:, 4:8, :],
                                    in1=u3[:, 4:8, :], op=OP.mult)
        poolU_cm.__exit__(None, None, None)

        # ---------------- Phase B: selective scan (block-outer) ----------------
        with tc.tile_pool(name="scan", bufs=2) as sp, \
             tc.tile_pool(name="quadp", bufs=1) as quadp, \
             tc.tile_pool(name="scanA", bufs=2) as spA, \
             tc.tile_pool(name="scan3", bufs=2) as sp3, \
             tc.tile_pool(name="oproj", bufs=1) as op_pool, \
             tc.tile_pool(name="osbp", bufs=3) as osbp, \
             tc.tile_pool(name="oc", bufs=1) as oc_pool, \
             tc.tile_pool(name="op_ps", bufs=4, space="PSUM") as op_ps:
            wco_r = wco.ap().rearrange("(k p) m -> p k m", p=128)
            outp_r = outp.ap().rearrange("b (m p) t -> b p m t", p=128)
            for blk in range(NB):
                g0 = blk * g_blk
                quad = None
                for n in range(NST):
                    brep = sp3.tile([128, L], scan_dt, tag="brep")
                    nc.sync.dma_start(out=brep[:], in_=cc_bf[n: n + 1, :].to_broadcast((128, L)))
                    crep = sp3.tile([128, L], scan_dt, tag="crep")
                    nc.sync.dma_start(out=crep[:], in_=cc_bf[16 + n: 17 + n, :].to_broadcast((128, L)))
                    dA = spA.tile([128, FB], scan_dt, tag="dA")
                    if structured_a:
                        nc.scalar.activation(
                            out=dA[:].rearrange("p (g t) -> p g t", g=g_blk),
                            in_=delta[:, g0:g0 + g_blk, :],
                            func=AF.Exp, bias=0.0, scale=At[:, 0, n:n + 1])
                    else:
                        for gg in range(g_blk):
                            nc.scalar.activation(
                                out=dA[:, gg * L:(gg + 1) * L],
                                in_=delta[:, g0 + gg, :],
                                func=AF.Exp, bias=0.0, scale=At[:, g0 + gg, n:n + 1])
                    dAz = bass.AP(tensor=dA.tensor, offset=dA.offset,
                                  ap=[[dA.ap[0][0], 128], [L, g_blk], [1, 1]])
                    nc.gpsimd.memset(dAz, 0.0)
                    dBu = sp.tile([128, FB], scan_dt, tag="dBu")
                    bap = bass.AP(tensor=brep.tensor, offset=brep.offset,
                                  ap=[[brep.ap[0][0], 128], [0, g_blk], [1, L]])
                    dbu_eng = nc.gpsimd if (n % 2 == 0) else nc.vector
                    dbu_eng.tensor_tensor(
                        out=dBu[:].rearrange("p (g t) -> p g t", g=g_blk),
                        in0=du[:, g0:g0 + g_blk, :], in1=bap, op=OP.mult)
                    h = sp.tile([128, FB], scan_dt, tag="h")
                    nc.vector.tensor_tensor_scan(h[:], dA[:], dBu[:], 0.0, OP.mult, OP.add)
                    # hc computed in place over h
                    cap = bass.AP(tensor=crep.tensor, offset=crep.offset,
                                  ap=[[crep.ap[0][0], 128], [0, g_blk], [1, L]])
                    ph = n % 4
                    hc_eng = nc.gpsimd if ph in (0, 1) else nc.vector
                    h3 = h[:].rearrange("p (g t) -> p g t", g=g_blk)
                    hc_eng.tensor_tensor(out=h3, in0=h3, in1=cap, op=OP.mult)
                    if ph == 0:
                        quad = quadp.tile([128, FB], scan_dt, tag="quad")
                        nc.gpsimd.tensor_copy(quad[:], h[:])
                    elif ph == 1:
                        nc.gpsimd.tensor_tensor(out=quad[:], in0=quad[:], in1=h[:], op=OP.add)
                    else:
                        nc.vector.tensor_tensor(out=quad[:], in0=quad[:], in1=h[:], op=OP.add)
                    if ph == 3:
                        nc.vector.tensor_tensor(
                            out=yac[:, g0:g0 + g_blk, :],
                            in0=yac[:, g0:g0 + g_blk, :],
                            in1=quad[:].rearrange("p (g t) -> p g t", g=g_blk), op=OP.add)
                # gate this block and emit its partial output projection
                ygb = op_pool.tile([128, g_blk, L], BF16, tag="ygb")
                nc.vector.tensor_tensor(out=ygb[:], in0=yac[:, g0:g0 + g_blk, :],
                                        in1=szg[:, g0:g0 + g_blk, :], op=OP.mult)
                for mc in range(2):
                    wc = oc_pool.tile([128, 8, 512], BF16, tag="wc")
                    nc.gpsimd.dma_start(out=wc[:], in_=wco_r[:, :, mc * 512:(mc + 1) * 512])
                    for mm in range(4):
                        m = mc * 4 + mm
                        osl = osbp.tile([128, L], F32, tag="osl")
                        for tn in range(NT):
                            ps = op_ps.tile([128, TN], F32, tag="ps_o")
                            for kk in range(g_blk):
                                nc.tensor.matmul(ps[:], wc[:, g0 + kk, mm * 128:(mm + 1) * 128],
                                                 ygb[:, kk, tn * TN:(tn + 1) * TN],
                                                 start=(kk == 0), stop=(kk == g_blk - 1))
                            nc.scalar.copy(out=osl[:, tn * TN:(tn + 1) * TN], in_=ps[:])
                        nc.sync.dma_start(out=outp_r[blk, :, m, :], in_=osl[:])
        poolD_cm.__exit__(None, None, None)
        szgp_cm.__exit__(None, None, None)
        yacp_cm.__exit__(None, None, None)
        consts_cm.__exit__(None, None, None)

    nc.compile()
    return nc


def _prep_core_inputs(inputs, b, d, h):
    pref = "f_" if d == 0 else "b_"
    g = lambda k: np.asarray(inputs[pref + k], dtype=np.float32)
    x = np.asarray(inputs["x"], dtype=np.float32)[b]
    if d == 1:
        x = x[::-1]
    sl = slice(h * DH, (h + 1) * DH)

    Win = g("Win")
    w_in = np.concatenate([Win[sl].T, Win[2048 + h * DH: 2048 + (h + 1) * DH].T], axis=1)
    convw = g("convw")[sl]
    convb_eff = g("convb")[sl] + g("bin")[sl] * convw.sum(-1)
    pg = lambda v: np.ascontiguousarray(v.reshape(NG, 128).T)
    pg3 = lambda v: np.ascontiguousarray(v.reshape(NG, 128, -1).transpose(1, 0, 2))
    A = -np.exp(g("Alog")[sl])
    proj_W = np.asarray(inputs["proj_W"], dtype=np.float32)
    Pd = proj_W[:, d * DM:(d + 1) * DM]
    wco = (Pd @ g("Wout"))[:, sl].T
    return {
        "xT": np.ascontiguousarray(x.T),
        "w_in": np.ascontiguousarray(w_in),
        "wx": np.ascontiguousarray(g("Wx")[:, sl].T),
        "wdt": np.ascontiguousarray(g("Wdt")[sl].T),
        "wco": np.ascontiguousarray(wco),
        "convw": pg3(convw),
        "convb": pg(convb_eff),
        "binz": pg(g("bin")[2048 + h * DH: 2048 + (h + 1) * DH]),
        "bdt": pg(g("bdt")[sl]),
        "A_": pg3(A),
        "dsk": pg(g("Dsk")[sl]),
    }


def _check_structured_a(inputs):
    ar = np.log(np.arange(1, NST + 1, dtype=np.float32))
    for pref in ("f_", "b_"):
        Alog = np.asarray(inputs[pref + "Alog"], dtype=np.float32)
        if not np.allclose(Alog, np.broadcast_to(ar, Alog.shape), atol=1e-5):
            return False
    return True


_CACHE = {}


def _get_nc(structured_a):
    key = ("v1", structured_a)
    if key not in _CACHE:
        _CACHE[key] = _build(structured_a)
    return _CACHE[key]


def kernel(**inputs):
    from concourse.bass_utils import run_bass_kernel_spmd

    nc = _get_nc(_check_structured_a(inputs))
    in_maps = []
    for c in range(N_CORES):
        b, d, h = c >> 2, (c >> 1) & 1, c & 1
        in_maps.append(_prep_core_inputs(inputs, b, d, h))
    res = run_bass_kernel_spmd(nc, in_maps, list(range(N_CORES)))
    partials = [res.results[c]["outp"].sum(axis=0) for c in range(N_CORES)]

    B = 2
    out = np.zeros((B, L, DM), np.float32)
    for b in range(B):
        for d in range(2):
            s = (partials[b * 4 + d * 2 + 0] + partials[b * 4 + d * 2 + 1]).T
            if d == 1:
                s = s[::-1]
            out[b] += s
    proj_W = np.asarray(inputs["proj_W"], dtype=np.float32)
    bias = (np.asarray(inputs["f_bout"], dtype=np.float32) @ proj_W[:, :DM].T
            + np.asarray(inputs["b_bout"], dtype=np.float32) @ proj_W[:, DM:].T
            + np.asarray(inputs["proj_b"], dtype=np.float32))
    return out + bias
